# revision 32
# baseline (speedup 1.0000x reference)
"""Two-layer GCN (PyG GCNConv x2 + leaky_relu(0.2)) on 8 trn2 NeuronCores.

Distribution strategy (dst-sharded graph parallel, bf16 datapath):
  - Nodes split 8 ways by dst; core c owns dsts [c*NLOC, (c+1)*NLOC).
  - Self-loops appended as edges.  Symmetric norm FACTORIZED:
    w_e = dinv[src]*dinv[dst]; the src factor rides per-edge in layer 1
    (P = P0 * a_e) and is pre-scaled into the AllGather'd h1 tables for
    layer 2 (folded into the Prelu activation's per-partition scale);
    the dst factor is applied at per-node epilogues (Prelu scale = dinv^2,
    layer-2 output scale = dinv), with rsqrt computed ON DEVICE from
    exact integer degrees.
  - Aggregation = one-hot matmuls on TensorE: per 128-edge chunk,
    lhsT = messages G (stationary), rhs = one-hot P streamed from host
    as pure {0,1} structure (layer 2: used directly, ZERO vector-engine
    work; layer 1: single multiply by a_e per chunk batch).
  - Layer-1 messages (x[src], bf16) host-prelaid in chunk-slot order and
    streamed (HWDGE).  Layer-2 messages gathered on device (gpsimd
    dma_gather) from 2 AllGather'd h1 tables in PAIR layout (512-byte
    rows = 2 nodes), halving descriptor-generation cost; each chunk does
    two matmuls (even/odd parity halves) with parity-split one-hot
    streams.
  - GEMMs per 128-node tile: h1 = Prelu_{0.2}((s1 @ W1)*dinv^2 + b1*...)
    (bias via rank-1 matmul with sqrt(deg) compensation), out = s2 @ W2.
"""

import os
import sys

import numpy as np

sys.path.insert(0, "/opt/trn_rl_repo")

P = 128
NCORES = 8
SLAB = 512       # dsts per PSUM slab (one 2KB fp32 bank)
WIN = 64         # narrow-chunk P width
NQ = 4           # gather-table quarters (int16 index range)
STEP_FRAC = 1.0  # target per-core edges per scheduled chunk / 128
PBATCH = 8       # narrow chunks per batched DVE P-scale op


# ---------------------------------------------------------------------------
# host-side structure prep
# ---------------------------------------------------------------------------

def _chunk_schedule(dl_pooled, n_max):
    """Shared window bases for one (slab, q) group from pooled local dsts."""
    if n_max == 0:
        return np.zeros(1, np.int64)
    step = max(1, int(P * STEP_FRAC))
    K = max(1, -(-n_max // step))
    npool = len(dl_pooled)
    bases = []
    prev = 0
    for k in range(K):
        b = int(dl_pooled[min(int(k * npool / K), npool - 1)]) if npool else 0
        b = max(prev if k else 0, b - 8)           # small low-side margin
        if k:
            b = min(b, prev + WIN)                 # reachability clamp
            b = max(b, prev)                       # monotone
        b = min(b, SLAB - WIN)
        bases.append(b)
        prev = b
    return np.asarray(bases, np.int64)


def _fill_core(dl, bases):
    """Greedy fill of one core's sorted dsts into scheduled windows."""
    n = len(dl)
    out = []
    leftover = []
    ptr = 0
    for b in bases:
        lo = ptr + int(np.searchsorted(dl[ptr:], b))
        if lo > ptr:
            leftover.extend(range(ptr, lo))        # below-window stragglers
        hi = lo + int(np.searchsorted(dl[lo:], b + WIN))
        j = min(lo + P, hi)
        out.append((lo, j, int(b)))
        ptr = j
    leftover.extend(range(ptr, n))
    return out, leftover


def host_prep(x, edge_index):
    n_nodes = x.shape[0]
    nloc = n_nodes // NCORES
    qsz = nloc // NQ
    nslab = -(-nloc // SLAB)
    ntile = -(-nloc // P)
    src = np.asarray(edge_index[0], np.int64)
    dst = np.asarray(edge_index[1], np.int64)

    deg = np.bincount(dst, minlength=n_nodes).astype(np.int64)
    srcA = np.concatenate([src, np.arange(n_nodes, dtype=np.int64)])
    dstA = np.concatenate([dst, np.arange(n_nodes, dtype=np.int64)])
    degp = deg + 1                                  # exact ints (< 2^24)

    core = dstA // nloc
    dloc = dstA % nloc
    slab = dloc // SLAB
    dsl = dloc - slab * SLAB
    q = (srcA % nloc) // qsz
    idxval = (qsz * (srcA // nloc) + (srcA % qsz)).astype(np.int32)

    order = np.lexsort((dsl, q, slab, core))
    srcA = srcA[order]; dsl = dsl[order]; slab = slab[order]
    q = q[order]; core = core[order]
    idxval = idxval[order]
    dga = degp[srcA].astype(np.float32)             # per-edge src degree

    key = (core * nslab + slab) * NQ + q
    starts = np.searchsorted(key, np.arange(NCORES * nslab * NQ + 1))

    def grp(c, s, qq):
        g = (c * nslab + s) * NQ + qq
        return int(starts[g]), int(starts[g + 1])

    # --- shared schedule per (slab, q): bases + total chunk count ---------
    sched = {}
    for s in range(nslab):
        for qq in range(NQ):
            segs = [grp(c, s, qq) for c in range(NCORES)]
            pooled = np.sort(np.concatenate([dsl[a:b] for a, b in segs]))
            n_max = max(b - a for a, b in segs)
            bases = _chunk_schedule(pooled, n_max)
            fills = []
            ov_max = 0
            for c in range(NCORES):
                a, b = segs[c]
                f, lo = _fill_core(dsl[a:b], bases)
                fills.append((a, f, lo))
                ov_max = max(ov_max, -(-len(lo) // P))
            sched[(s, qq)] = (bases, fills, ov_max)

    # chunk meta in program order
    prog = []
    for s in range(nslab):
        for qq in range(NQ):
            bases, _, ov_max = sched[(s, qq)]
            for k in range(len(bases)):
                if qq == 0 and k == 0:
                    prog.append((s, qq, "wide", 0))
                else:
                    prog.append((s, qq, "norm", int(bases[k])))
            for _ in range(ov_max):
                prog.append((s, qq, "ovfl", 0))
    nch = len(prog)

    # wide/ovfl chunk -> compact column index
    wlist = [k for k, pr in enumerate(prog) if pr[2] in ("wide", "ovfl")]
    wmap = {k: i for i, k in enumerate(wlist)}
    NW = max(8, len(wlist))

    # --- per-core slot arrays --------------------------------------------
    per_core = []
    for c in range(NCORES):
        slots_src = np.zeros(nch * P, np.int64)
        a_off = np.full((nch, P), -1.0, np.float32)
        a_dga = np.ones((nch, P), np.float32)
        a_idx = np.zeros(nch * P, np.int32)
        ci = 0
        for s in range(nslab):
            for qq in range(NQ):
                bases, fills, ov_max = sched[(s, qq)]
                a, f, lo = fills[c]
                for k in range(len(bases)):
                    i0, i1, b = f[k]
                    m = i1 - i0
                    if m > 0:
                        sl = slice(ci * P, ci * P + m)
                        rows = slice(a + i0, a + i1)
                        slots_src[sl] = srcA[rows]
                        a_idx[sl] = idxval[rows]
                        a_dga[ci, :m] = dga[rows]
                        if qq == 0 and k == 0:
                            a_off[ci, :m] = dsl[rows]
                        else:
                            a_off[ci, :m] = dsl[rows] - b
                    ci += 1
                for o in range(ov_max):
                    idxs_ = lo[o * P:(o + 1) * P]
                    m = len(idxs_)
                    if m > 0:
                        rows = a + np.asarray(idxs_, np.int64)
                        sl = slice(ci * P, ci * P + m)
                        slots_src[sl] = srcA[rows]
                        a_idx[sl] = idxval[rows]
                        a_dga[ci, :m] = dga[rows]
                        a_off[ci, :m] = dsl[rows]
                    ci += 1
        assert ci == nch
        per_core.append(dict(slots_src=slots_src, a_off=a_off, a_dga=a_dga,
                             a_idx=a_idx))

    placed = sum((pc["a_off"] >= 0).sum() for pc in per_core)
    assert placed == len(srcA), (placed, len(srcA))

    # per-node degree tables (exact ints)
    degb_cols = []
    for c in range(NCORES):
        dp = degp[c * nloc:(c + 1) * nloc].astype(np.float32)
        col = np.ones((ntile * P,), np.float32)
        col[:nloc] = dp
        degb_cols.append(col.reshape(ntile, P).T.copy())   # [P, ntile]

    return dict(n_nodes=n_nodes, nloc=nloc, qsz=qsz, nslab=nslab, nch=nch,
                ntile=ntile, prog=prog, per_core=per_core, wlist=wlist,
                wmap=wmap, NW=NW, degb_cols=degb_cols)


# ---------------------------------------------------------------------------
# device program
# ---------------------------------------------------------------------------

def build_program(meta):
    import concourse.bacc as bacc
    import concourse.bass as bass
    import concourse.tile as tile
    from concourse import mybir

    nloc, qsz, nslab, nch = meta["nloc"], meta["qsz"], meta["nslab"], meta["nch"]
    ntile, NW = meta["ntile"], meta["NW"]
    prog, wmap = meta["prog"], meta["wmap"]
    tabrows = qsz * NCORES
    f32 = mybir.dt.float32
    f16 = mybir.dt.bfloat16
    AF = mybir.ActivationFunctionType

    nc = bacc.Bacc("TRN2", target_bir_lowering=False, debug=False,
                   num_devices=NCORES, num_swdge_queues=4)

    g1 = nc.dram_tensor("g1", [P, nch, P], f16, kind="ExternalInput")
    p0a = nc.dram_tensor("p0a", [P, nch, WIN], f16, kind="ExternalInput")
    idxs = nc.dram_tensor("idxs", [P, nch * P // 16], mybir.dt.int16,
                          kind="ExternalInput")
    dega = nc.dram_tensor("dega", [P, nch], f32, kind="ExternalInput")
    offw_f = nc.dram_tensor("offw_f", [P, NW], f32, kind="ExternalInput")
    degb_c = nc.dram_tensor("degb_c", [P, ntile], f32, kind="ExternalInput")
    degaw = nc.dram_tensor("degaw", [P, NW], f32, kind="ExternalInput")
    sqscr = nc.dram_tensor("sqscr", [1, ntile * P], f16)
    w1_t = nc.dram_tensor("w1", [P, P], f16, kind="ExternalInput")
    b1_t = nc.dram_tensor("b1", [1, P], f16, kind="ExternalInput")
    w2_t = nc.dram_tensor("w2", [P, 3], f16, kind="ExternalInput")
    b2_t = nc.dram_tensor("b2", [1, 3], f16, kind="ExternalInput")
    out_t = nc.dram_tensor("out", [nloc, 3], f32, kind="ExternalOutput")

    h1_in = [nc.dram_tensor(f"h1_in{qq}", [qsz, P], f16) for qq in range(NQ)]
    h1_tab = [nc.dram_tensor(f"h1_tab{qq}", [tabrows, P], f16,
                             addr_space="Shared") for qq in range(NQ)]

    # chunk ranges per (s, qq)
    group_of = {}
    pos = 0
    for s in range(nslab):
        for qq in range(NQ):
            k0 = pos
            while pos < nch and prog[pos][0] == s and prog[pos][1] == qq:
                pos += 1
            group_of[(s, qq)] = (k0, pos)
    assert pos == nch

    with tile.TileContext(nc) as tc:
        with tc.tile_pool(name="const", bufs=1) as cpool, \
             tc.tile_pool(name="stsb", bufs=1) as spool, \
             tc.tile_pool(name="g1buf", bufs=2) as g1pool, \
             tc.tile_pool(name="g2buf", bufs=3) as g2pool, \
             tc.tile_pool(name="pabuf", bufs=3) as papool, \
             tc.tile_pool(name="pmbuf", bufs=8) as ppool, \
             tc.tile_pool(name="evbuf", bufs=6) as epool, \
             tc.tile_pool(name="psum", bufs=4, space="PSUM") as pspool, \
             tc.tile_pool(name="psg", bufs=2, space="PSUM") as psg, \
             tc.tile_pool(name="psg2", bufs=2, space="PSUM") as psg2:

            # ---- constants / structure loads ----
            dega_sb = cpool.tile([P, nch], f32)
            nc.sync.dma_start(out=dega_sb[:], in_=dega[:])
            nc.vector.reciprocal(out=dega_sb[:], in_=dega_sb[:])
            a16 = cpool.tile([P, nch], f16)
            nc.scalar.activation(out=a16[:], in_=dega_sb[:], func=AF.Sqrt)
            # compact fp32 src factor for the wide-chunk builds
            degaw_sb = cpool.tile([P, NW], f32)
            nc.sync.dma_start(out=degaw_sb[:], in_=degaw[:])
            nc.vector.reciprocal(out=degaw_sb[:], in_=degaw_sb[:])
            a32w = cpool.tile([P, NW], f32)
            nc.scalar.activation(out=a32w[:], in_=degaw_sb[:], func=AF.Sqrt)

            offw_f_sb = cpool.tile([P, NW], f32)
            nc.sync.dma_start(out=offw_f_sb[:], in_=offw_f[:])

            def wbcast(t, wi, width):
                ap = t[:, wi:wi + 1]
                return bass.AP(t.tensor, ap.offset,
                               [list(ap.ap[0]), [0, width]])

            degb_c_sb = cpool.tile([P, ntile], f32)
            nc.sync.dma_start(out=degb_c_sb[:], in_=degb_c[:])
            dv2_col = cpool.tile([P, ntile], f32)
            nc.vector.reciprocal(out=dv2_col[:], in_=degb_c_sb[:])
            dinv_col = cpool.tile([P, ntile], f32)
            nc.scalar.activation(out=dinv_col[:], in_=dv2_col[:],
                                 func=AF.Sqrt)

            # sqrt(deg) in row layout via DMA transpose through DRAM scratch
            sq_col = cpool.tile([P, ntile], f16)
            nc.scalar.activation(out=sq_col[:], in_=degb_c_sb[:],
                                 func=AF.Sqrt)
            sq_out_ap = bass.AP(sqscr[:].tensor, 0, [[1, P], [P, ntile]])
            nc.sync.dma_start(out=sq_out_ap, in_=sq_col[:])
            sq16_row = cpool.tile([1, nloc], f16)
            nc.sync.dma_start(out=sq16_row[:], in_=sqscr[0:1, 0:nloc])

            iota_w = cpool.tile([P, SLAB], f32)
            nc.gpsimd.iota(iota_w[:], [[1, SLAB]], base=0, channel_multiplier=0,
                           allow_small_or_imprecise_dtypes=True)

            w1_sb = cpool.tile([P, P], f16)
            nc.sync.dma_start(out=w1_sb[:], in_=w1_t[:])
            b1_sb = cpool.tile([1, P], f16)
            nc.sync.dma_start(out=b1_sb[:], in_=b1_t[:])
            w2_sb = cpool.tile([P, 3], f16)
            nc.sync.dma_start(out=w2_sb[:], in_=w2_t[:])
            b2_sb = cpool.tile([1, 3], f16)
            nc.sync.dma_start(out=b2_sb[:], in_=b2_t[:])
            alpha_sb = cpool.tile([P, 1], f32)
            nc.vector.memset(alpha_sb[:], 0.2)

            st_sb = spool.tile([P, nloc], f16, tag="stT")  # s1T (half-normed)
            out_acc = spool.tile([P, ntile, 4], f32, tag="outacc")
            nc.vector.memset(out_acc[:], 0.0)

            def gemm1_tile(t):
                r0 = t * P
                m = min(P, nloc - r0)
                hps = psg.tile([P, P], f32, tag="gemm_ps")
                nc.tensor.matmul(out=hps[:m, :], lhsT=st_sb[:, r0:r0 + m],
                                 rhs=w1_sb[:], start=True, stop=False)
                nc.tensor.matmul(out=hps[:m, :], lhsT=sq16_row[:, r0:r0 + m],
                                 rhs=b1_sb[:], start=False, stop=True)
                h_sb = epool.tile([P, P], f16, tag="h1t")
                # Prelu(x*dinv^2) = dinv^2*Prelu(x); table lands pre-scaled
                # by dinv[node] beyond the true h1 (dst factor + L2 src
                # factor merge into one reciprocal).
                nc.scalar.activation(out=h_sb[:m, :], in_=hps[:m, :],
                                     func=AF.Prelu,
                                     scale=dv2_col[:m, t:t + 1],
                                     alpha=alpha_sb[:m, 0:1])
                r = r0
                while r < r0 + m:
                    qq = r // qsz
                    rq = r - qq * qsz
                    span = min(r0 + m - r, qsz - rq)
                    nc.sync.dma_start(
                        out=h1_in[qq][rq:rq + span, :],
                        in_=h_sb[r - r0:r - r0 + span, :])
                    r += span

            def do_allgather(qq):
                nc.gpsimd.collective_compute(
                    "AllGather", mybir.AluOpType.bypass,
                    replica_groups=[list(range(NCORES))],
                    ins=[h1_in[qq][:]], outs=[h1_tab[qq][:]])

            ntile_l = ntile
            ag_after_tile = [-(-((q + 1) * qsz) // P) - 1 for q in range(NQ)]

            def a16_bcast(k, nb):
                ap = a16[:, k:k + nb]
                return bass.AP(a16.tensor, ap.offset,
                               [list(ap.ap[0]), list(ap.ap[1]), [0, WIN]])

            # ---- one aggregation layer slab ----
            def agg_layer_slab(layer, s, qsel=None):
                    wlo = s * SLAB
                    wid = min(SLAB, nloc - wlo)
                    acc = pspool.tile([P, SLAB], f32, tag="agg_ps")
                    if layer == 1:
                        nc.vector.memset(acc[:], 0.0)
                    for qq in ([qsel] if qsel is not None else range(NQ)):
                        k0, k1 = group_of[(s, qq)]
                        kn = k1 - k0
                        if layer == 0:
                            gt = g1pool.tile([P, kn, P], f16, tag="gt1")
                            nc.sync.dma_start(out=gt[:], in_=g1[:, k0:k1, :])
                            pa = papool.tile([P, kn, WIN], f16, tag="pa")
                            nc.sync.dma_start(out=pa[:], in_=p0a[:, k0:k1, :])
                        else:
                            ist = g2pool.tile([P, kn * P // 16],
                                              mybir.dt.int16, tag="ist")
                            nc.sync.dma_start(
                                out=ist[:],
                                in_=idxs[:, k0 * P // 16: k1 * P // 16])
                            gt = g2pool.tile([P, kn, P], f16, tag="gt2")
                            ni = kn * P
                            nc.gpsimd.dma_gather(
                                gt[:], h1_tab[qq][:], ist[:],
                                ni, ni, P, single_packet=False,
                                queue_num=(s + qq) % 4)
                            pa = papool.tile([P, kn, WIN], f16, tag="pa")
                            nc.sync.dma_start(out=pa[:], in_=p0a[:, k0:k1, :])
                        # P scale (L1 only) + matmuls
                        k = k0
                        while k < k1:
                            kind = prog[k][2]
                            if kind in ("wide", "ovfl"):
                                wi = wmap[k]
                                pm = ppool.tile([P, SLAB], f16,
                                                tag="pwide")
                                nc.vector.tensor_tensor(
                                    out=pm[:], in0=iota_w[:],
                                    in1=wbcast(offw_f_sb, wi, SLAB),
                                    op=mybir.AluOpType.is_equal)
                                if layer == 0:
                                    last = (qq == NQ - 1 and k == k1 - 1)
                                    first = (kind == "wide")
                                    nc.vector.tensor_tensor(
                                        out=pm[:], in0=pm[:],
                                        in1=wbcast(a32w, wi, SLAB),
                                        op=mybir.AluOpType.mult)
                                    nc.tensor.matmul(
                                        out=acc[:], lhsT=gt[:, k - k0, :],
                                        rhs=pm[:], start=first, stop=last)
                                else:
                                    last = (k == k1 - 1)
                                    nc.tensor.matmul(
                                        out=acc[:], lhsT=gt[:, k - k0, :],
                                        rhs=pm[:], start=False, stop=last)
                                k += 1
                            elif layer == 0:
                                nb = 1
                                while (nb < PBATCH and k + nb < k1
                                       and prog[k + nb][2] == "norm"):
                                    nb += 1
                                pm = ppool.tile([P, nb, WIN], f16,
                                                tag=f"pn{nb}")
                                nc.vector.tensor_tensor(
                                    out=pm[:], in0=pa[:, k - k0:k - k0 + nb, :],
                                    in1=a16_bcast(k, nb),
                                    op=mybir.AluOpType.mult)
                                for j in range(nb):
                                    base = prog[k + j][3]
                                    last = (qq == NQ - 1 and k + j == k1 - 1)
                                    nc.tensor.matmul(
                                        out=acc[:, base:base + WIN],
                                        lhsT=gt[:, k + j - k0, :],
                                        rhs=pm[:, j, :],
                                        start=False, stop=last)
                                k += nb
                            else:
                                base = prog[k][3]
                                kj = k - k0
                                last = (k == k1 - 1)
                                nc.tensor.matmul(
                                    out=acc[:, base:base + WIN],
                                    lhsT=gt[:, kj, :],
                                    rhs=pa[:, kj, :],
                                    start=False, stop=last)
                                k += 1
                    if layer == 0:
                        nc.vector.tensor_copy(out=st_sb[:, wlo:wlo + wid],
                                              in_=acc[:, :wid])
                        return None
                    ev = epool.tile([P, SLAB], f16, tag="l2ev")
                    nc.vector.tensor_copy(out=ev[:, :wid], in_=acc[:, :wid])

                    def epilogue(s=s, qsel=qsel, ev=ev, wid=wid):
                        t0 = (s * SLAB) // P
                        for tt in range(t0, min(t0 + SLAB // P, ntile)):
                            c0 = tt * P - s * SLAB
                            m = min(P, nloc - tt * P)
                            ops = psg2.tile([P, 4], f32, tag="gemm2_ps")
                            nc.tensor.matmul(out=ops[:m, :3],
                                             lhsT=ev[:, c0:c0 + m],
                                             rhs=w2_sb[:], start=True,
                                             stop=(qsel != 0))
                            if qsel == 0:
                                nc.tensor.matmul(
                                    out=ops[:m, :3],
                                    lhsT=sq16_row[:, tt * P:tt * P + m],
                                    rhs=b2_sb[:], start=False, stop=True)
                            nc.vector.scalar_tensor_tensor(
                                out=out_acc[:m, tt, :3],
                                in0=ops[:m, :3],
                                scalar=dinv_col[:m, tt:tt + 1],
                                in1=out_acc[:m, tt, :3],
                                op0=mybir.AluOpType.mult,
                                op1=mybir.AluOpType.add)
                    return epilogue

            # ====== layer 1 + layer 2 interleaved emission ======
            l2_queue = [(qq, s) for qq in range(NQ) for s in range(nslab)]
            l2_pos = 0
            pend_ep = []

            def emit_l2(qq2, s2):
                ep = agg_layer_slab(1, s2, qsel=qq2)
                pend_ep.append(ep)
                if len(pend_ep) > 2:
                    pend_ep.pop(0)()
            ag_emitted = 0
            done_tiles = 0
            for s in range(nslab):
                agg_layer_slab(0, s)
                cover = min(ntile_l, ((s + 1) * SLAB) // P) if s < nslab - 1 \
                    else ntile_l
                while done_tiles < cover:
                    gemm1_tile(done_tiles)
                    while ag_emitted < NQ and ag_after_tile[ag_emitted] == done_tiles:
                        do_allgather(ag_emitted)
                        ag_emitted += 1
                    done_tiles += 1
                # pace L2 groups whose table is already in flight
                budget = 3
                while (budget > 0 and l2_pos < len(l2_queue)
                       and l2_queue[l2_pos][0] < ag_emitted):
                    qq2, s2 = l2_queue[l2_pos]
                    emit_l2(qq2, s2)
                    l2_pos += 1
                    budget -= 1
            assert done_tiles == ntile_l and ag_emitted == NQ
            while l2_pos < len(l2_queue):
                qq2, s2 = l2_queue[l2_pos]
                emit_l2(qq2, s2)
                l2_pos += 1
            for ep in pend_ep:
                ep()

            for t in range(ntile):
                r0 = t * P
                m = min(P, nloc - r0)
                nc.sync.dma_start(out=out_t[r0:r0 + m, :],
                                  in_=out_acc[:m, t, :3])

    nc.compile()
    return nc


# ---------------------------------------------------------------------------
# entry point
# ---------------------------------------------------------------------------

def _wrap_idx(vals):
    """int16 gather index layout: slot i -> [i%16, i//16], tiled to 128 rows."""
    v = vals.reshape(-1, 16).T  # [16, n/16]
    return np.tile(v, (8, 1)).astype(np.int16)


def kernel(x, W1, b1, W2, b2, edge_index, _profile=None):
    import ml_dtypes
    bf16 = ml_dtypes.bfloat16
    x = np.asarray(x, np.float32)
    x16 = x.astype(bf16)
    meta = host_prep(x, edge_index)
    nloc, nch, NW = meta["nloc"], meta["nch"], meta["NW"]
    wlist = meta["wlist"]

    from concourse.bass_utils import run_bass_kernel_spmd
    nc = build_program(meta)

    jwin = np.arange(WIN, dtype=np.float32)
    in_maps = []
    for c in range(NCORES):
        pc = meta["per_core"][c]
        off = pc["a_off"]                      # [nch, P]
        g1 = np.ascontiguousarray(
            x16[pc["slots_src"].reshape(nch, P)].transpose(1, 0, 2))
        # one-hot streams [P, nch, WIN] (pure structure)
        oh = (off[:, :, None] == jwin[None, None, :])          # [nch,P,WIN]
        p0a = np.ascontiguousarray(
            oh.transpose(1, 0, 2)).astype(bf16)
        # wide-chunk compact offset tables
        offw_f = np.full((NW, P), -1.0, np.float32)
        degaw = np.ones((NW, P), np.float32)
        for wi, k in enumerate(wlist):
            offw_f[wi] = off[k]
            degaw[wi] = pc["a_dga"][k]
        in_maps.append(dict(
            g1=g1,
            p0a=p0a,
            idxs=_wrap_idx(pc["a_idx"]),
            dega=np.ascontiguousarray(pc["a_dga"].T),
            offw_f=np.ascontiguousarray(offw_f.T),
            degaw=np.ascontiguousarray(degaw.T),
            degb_c=meta["degb_cols"][c],
            w1=np.asarray(W1, np.float32).astype(bf16),
            b1=np.asarray(b1, np.float32).reshape(1, P).astype(bf16),
            w2=np.asarray(W2, np.float32).astype(bf16),
            b2=np.asarray(b2, np.float32).reshape(1, 3).astype(bf16),
        ))

    res = run_bass_kernel_spmd(nc, in_maps, list(range(NCORES)),
                               trace=_profile is not None)
    if _profile is not None:
        _profile["exec_time_ns"] = res.exec_time_ns
    out = np.concatenate([res.results[c]["out"] for c in range(NCORES)], axis=0)
    return out.astype(np.float32)


# revision 33
# speedup vs baseline: 1.1155x; 1.1155x over previous
"""Two-layer GCN (PyG GCNConv x2 + leaky_relu(0.2)) on 8 trn2 NeuronCores.

Distribution strategy (dst-sharded graph parallel):
  - Nodes split 8 ways by dst; core c owns dsts [c*NLOC, (c+1)*NLOC).
  - Self-loops appended as edges; full symmetric norm folded into per-edge
    weights w_e = rsqrt((deg[src]+1)*(deg[dst]+1)), computed ON DEVICE from
    integer degree products (uploaded as exact f32 ints).
  - Aggregation = one-hot matmuls on TensorE: per 128-edge chunk,
    lhsT = messages G [128e x 128f] (stationary), rhs = P [128e x 64d]
    (P[e,j] = w_e * (dstoff_e == j), built on VectorE by iota compare),
    accumulating s^T [128f x 512d] PSUM slabs.
  - Chunk dst-windows use SHARED bases (pooled quantiles) so PSUM offsets
    are program literals valid for every core; per-core leftovers go to
    shared full-width overflow chunks.
  - Layer-1 messages (x[src]) are host-prelaid in chunk-slot order and
    streamed sequentially (HWDGE).  Layer-2 messages (h1[src]) are gathered
    on device (gpsimd dma_gather) from 4 AllGather'd h1 tables of
    NLOC*2 = 25000 rows each (int16-index-safe, no offset views).
  - GEMMs per 128-node tile: h1 = Prelu_{0.2}(s1 @ W1 + b1) (bias via rank-1
    matmul into the same PSUM group), out = s2 @ W2 + b2.
"""

import os
import sys

import numpy as np

sys.path.insert(0, "/opt/trn_rl_repo")

P = 128
NCORES = 8
SLAB = 512       # dsts per PSUM slab (one 2KB fp32 bank)
WIN = 64         # narrow-chunk P width
NQ = 4           # gather-table quarters (int16 index range)
STEP_FRAC = 1.0 # target per-core edges per scheduled chunk / 128
PBATCH = 8       # narrow chunks per batched DVE P-build op


# ---------------------------------------------------------------------------
# host-side structure prep
# ---------------------------------------------------------------------------

def _chunk_schedule(dl_pooled, n_max):
    """Shared window bases for one (slab, q) group from pooled local dsts.

    Returns monotone bases (step clamped to <= WIN) for K scheduled chunks.
    """
    if n_max == 0:
        return np.zeros(1, np.int64)
    step = max(1, int(P * STEP_FRAC))
    K = max(1, -(-n_max // step))
    npool = len(dl_pooled)
    bases = []
    prev = 0
    for k in range(K):
        b = int(dl_pooled[min(int(k * npool / K), npool - 1)]) if npool else 0
        b = max(prev if k else 0, b - 8)           # small low-side margin
        if k:
            b = min(b, prev + WIN)                 # reachability clamp
            b = max(b, prev)                       # monotone
        b = min(b, SLAB - WIN)
        bases.append(b)
        prev = b
    return np.asarray(bases, np.int64)


def _fill_core(dl, bases):
    """Greedy fill of one core's sorted dsts into scheduled windows.

    Returns list of (i0, i1, base) per scheduled chunk + leftover index list.
    """
    n = len(dl)
    out = []
    leftover = []
    ptr = 0
    for b in bases:
        lo = ptr + int(np.searchsorted(dl[ptr:], b))
        if lo > ptr:
            leftover.extend(range(ptr, lo))        # below-window stragglers
        hi = lo + int(np.searchsorted(dl[lo:], b + WIN))
        j = min(lo + P, hi)
        out.append((lo, j, int(b)))
        ptr = j
    leftover.extend(range(ptr, n))
    return out, leftover


def host_prep(x, edge_index):
    n_nodes = x.shape[0]
    nloc = n_nodes // NCORES
    qsz = nloc // NQ
    nslab = -(-nloc // SLAB)
    src = np.asarray(edge_index[0], np.int64)
    dst = np.asarray(edge_index[1], np.int64)

    deg = np.bincount(dst, minlength=n_nodes).astype(np.int64)
    srcA = np.concatenate([src, np.arange(n_nodes, dtype=np.int64)])
    dstA = np.concatenate([dst, np.arange(n_nodes, dtype=np.int64)])
    degp = deg + 1
    degprod = (degp[srcA] * degp[dstA]).astype(np.float32)  # exact (< 2^24)

    core = dstA // nloc
    dloc = dstA % nloc
    slab = dloc // SLAB
    dsl = dloc - slab * SLAB
    q = (srcA % nloc) // qsz
    idxval = (qsz * (srcA // nloc) + (srcA % qsz)).astype(np.int32)

    order = np.lexsort((dsl, q, slab, core))
    srcA = srcA[order]; dsl = dsl[order]; slab = slab[order]
    q = q[order]; core = core[order]
    idxval = idxval[order]; degprod = degprod[order]

    key = (core * nslab + slab) * NQ + q
    starts = np.searchsorted(key, np.arange(NCORES * nslab * NQ + 1))

    def grp(c, s, qq):
        g = (c * nslab + s) * NQ + qq
        return int(starts[g]), int(starts[g + 1])

    # --- shared schedule per (slab, q): bases + total chunk count ---------
    sched = {}
    for s in range(nslab):
        for qq in range(NQ):
            segs = [grp(c, s, qq) for c in range(NCORES)]
            pooled = np.sort(np.concatenate([dsl[a:b] for a, b in segs]))
            n_max = max(b - a for a, b in segs)
            bases = _chunk_schedule(pooled, n_max)
            fills = []
            ov_max = 0
            for c in range(NCORES):
                a, b = segs[c]
                f, lo = _fill_core(dsl[a:b], bases)
                fills.append((a, f, lo))
                ov_max = max(ov_max, -(-len(lo) // P))
            sched[(s, qq)] = (bases, fills, ov_max)

    # chunk meta in program order: (s, qq, kind, base) ; kind: 'wide' first
    # chunk of each slab (width SLAB, start=True), 'norm' width WIN,
    # 'ovfl' width SLAB.
    prog = []
    for s in range(nslab):
        for qq in range(NQ):
            bases, _, ov_max = sched[(s, qq)]
            for k in range(len(bases)):
                if qq == 0 and k == 0:
                    prog.append((s, qq, "wide", 0))
                else:
                    prog.append((s, qq, "norm", int(bases[k])))
            for _ in range(ov_max):
                prog.append((s, qq, "ovfl", 0))
    nch = len(prog)

    # --- per-core slot arrays --------------------------------------------
    per_core = []
    for c in range(NCORES):
        slots_src = np.zeros(nch * P, np.int64)
        a_off = np.full((nch, P), -1.0, np.float32)
        a_dpr = np.ones((nch, P), np.float32)
        a_idx = np.zeros(nch * P, np.int32)
        ci = 0
        for s in range(nslab):
            for qq in range(NQ):
                bases, fills, ov_max = sched[(s, qq)]
                a, f, lo = fills[c]
                for k in range(len(bases)):
                    i0, i1, b = f[k]
                    m = i1 - i0
                    if m > 0:
                        sl = slice(ci * P, ci * P + m)
                        rows = slice(a + i0, a + i1)
                        slots_src[sl] = srcA[rows]
                        a_idx[sl] = idxval[rows]
                        a_dpr[ci, :m] = degprod[rows]
                        if qq == 0 and k == 0:
                            a_off[ci, :m] = dsl[rows]
                        else:
                            a_off[ci, :m] = dsl[rows] - b
                    ci += 1
                for o in range(ov_max):
                    idxs = lo[o * P:(o + 1) * P]
                    m = len(idxs)
                    if m > 0:
                        rows = a + np.asarray(idxs, np.int64)
                        sl = slice(ci * P, ci * P + m)
                        slots_src[sl] = srcA[rows]
                        a_idx[sl] = idxval[rows]
                        a_dpr[ci, :m] = degprod[rows]
                        a_off[ci, :m] = dsl[rows]
                    ci += 1
        assert ci == nch
        per_core.append(dict(slots_src=slots_src, a_off=a_off, a_dpr=a_dpr,
                             a_idx=a_idx))

    # sanity: every edge placed exactly once
    placed = sum((pc["a_off"] >= 0).sum() for pc in per_core)
    assert placed == len(srcA), (placed, len(srcA))

    return dict(n_nodes=n_nodes, nloc=nloc, qsz=qsz, nslab=nslab, nch=nch,
                prog=prog, per_core=per_core)


# ---------------------------------------------------------------------------
# device program
# ---------------------------------------------------------------------------

def build_program(meta):
    import concourse.bacc as bacc
    import concourse.bass as bass
    import concourse.tile as tile
    from concourse import mybir

    nloc, qsz, nslab, nch = meta["nloc"], meta["qsz"], meta["nslab"], meta["nch"]
    prog = meta["prog"]
    tabrows = qsz * NCORES
    f32 = mybir.dt.float32
    f16 = mybir.dt.bfloat16

    nc = bacc.Bacc("TRN2", target_bir_lowering=False, debug=False,
                   num_devices=NCORES, num_swdge_queues=4)

    g1 = nc.dram_tensor("g1", [P, nch, P], f16, kind="ExternalInput")
    idxs = nc.dram_tensor("idxs", [P, nch * P // 16], mybir.dt.int16,
                          kind="ExternalInput")
    dstoff = nc.dram_tensor("dstoff", [P, nch], f32, kind="ExternalInput")
    dprod = nc.dram_tensor("dprod", [P, nch], f32, kind="ExternalInput")
    w1_t = nc.dram_tensor("w1", [P, P], f16, kind="ExternalInput")
    b1_t = nc.dram_tensor("b1", [1, P], f16, kind="ExternalInput")
    w2_t = nc.dram_tensor("w2", [P, 3], f16, kind="ExternalInput")
    b2_t = nc.dram_tensor("b2", [1, 3], f16, kind="ExternalInput")
    out_t = nc.dram_tensor("out", [nloc, 3], f32, kind="ExternalOutput")

    h1_in = [nc.dram_tensor(f"h1_in{qq}", [qsz, P], f16) for qq in range(NQ)]
    h1_tab = [nc.dram_tensor(f"h1_tab{qq}", [tabrows, P], f16,
                             addr_space="Shared") for qq in range(NQ)]

    # chunk ranges per (s, qq): [k0, k1) in program order + gather call list
    group_of = {}
    pos = 0
    for s in range(nslab):
        for qq in range(NQ):
            k0 = pos
            while pos < nch and prog[pos][0] == s and prog[pos][1] == qq:
                pos += 1
            group_of[(s, qq)] = (k0, pos)
    assert pos == nch

    ntile = -(-nloc // P)

    with tile.TileContext(nc) as tc:
        with tc.tile_pool(name="const", bufs=1) as cpool, \
             tc.tile_pool(name="stsb", bufs=1) as spool, \
             tc.tile_pool(name="gbuf", bufs=6) as gpool, \
             tc.tile_pool(name="pbuf", bufs=10) as ppool, \
             tc.tile_pool(name="evbuf", bufs=6) as epool, \
             tc.tile_pool(name="psum", bufs=4, space="PSUM") as pspool, \
             tc.tile_pool(name="psg", bufs=2, space="PSUM") as psg, \
             tc.tile_pool(name="psg2", bufs=2, space="PSUM") as psg2:

            # ---- constants / structure loads ----
            off_sb = cpool.tile([P, nch], f32)
            nc.sync.dma_start(out=off_sb[:], in_=dstoff[:])
            dpr_sb = cpool.tile([P, nch], f32)
            nc.sync.dma_start(out=dpr_sb[:], in_=dprod[:])
            rc_sb = cpool.tile([P, nch], f32)
            nc.vector.reciprocal(out=rc_sb[:], in_=dpr_sb[:])
            w_sb = cpool.tile([P, nch], f32)
            nc.scalar.activation(out=w_sb[:], in_=rc_sb[:],
                                 func=mybir.ActivationFunctionType.Sqrt)
            off16 = cpool.tile([P, nch], f16)
            nc.vector.tensor_copy(out=off16[:], in_=off_sb[:])
            w16 = cpool.tile([P, nch], f16)
            nc.vector.tensor_copy(out=w16[:], in_=w_sb[:])

            iota_w = cpool.tile([P, SLAB], f32)
            nc.gpsimd.iota(iota_w[:], [[1, SLAB]], base=0, channel_multiplier=0,
                           allow_small_or_imprecise_dtypes=True)
            iota_rep = cpool.tile([P, PBATCH, WIN], f16)
            for jj in range(PBATCH):
                nc.vector.tensor_copy(out=iota_rep[:, jj, :],
                                      in_=iota_w[:, :WIN])

            w1_sb = cpool.tile([P, P], f16)
            nc.sync.dma_start(out=w1_sb[:], in_=w1_t[:])
            b1_sb = cpool.tile([1, P], f16)
            nc.sync.dma_start(out=b1_sb[:], in_=b1_t[:])
            w2_sb = cpool.tile([P, 3], f16)
            nc.sync.dma_start(out=w2_sb[:], in_=w2_t[:])
            b2_sb = cpool.tile([1, 3], f16)
            nc.sync.dma_start(out=b2_sb[:], in_=b2_t[:])
            ones_sb = cpool.tile([1, P], f16)
            nc.vector.memset(ones_sb[:], 1.0)
            alpha_sb = cpool.tile([P, 1], f32)
            nc.vector.memset(alpha_sb[:], 0.2)

            st_sb = spool.tile([P, nloc], f16, tag="stT")  # s1T (layer 1)
            out_acc = spool.tile([P, ntile, 4], f32, tag="outacc")
            nc.vector.memset(out_acc[:], 0.0)

            def gemm1_tile(t):
                r0 = t * P
                m = min(P, nloc - r0)
                hps = psg.tile([P, P], f32, tag="gemm_ps")
                nc.tensor.matmul(out=hps[:m, :], lhsT=st_sb[:, r0:r0 + m],
                                 rhs=w1_sb[:], start=True, stop=False)
                nc.tensor.matmul(out=hps[:m, :], lhsT=ones_sb[:, :m],
                                 rhs=b1_sb[:], start=False, stop=True)
                h_sb = epool.tile([P, P], f16, tag="h1t")
                nc.scalar.activation(out=h_sb[:m, :], in_=hps[:m, :],
                                     func=mybir.ActivationFunctionType.Prelu,
                                     alpha=alpha_sb[:m, 0:1])
                r = r0
                while r < r0 + m:
                    qq = r // qsz
                    rq = r - qq * qsz
                    span = min(r0 + m - r, qsz - rq)
                    nc.sync.dma_start(
                        out=h1_in[qq][rq:rq + span, :],
                        in_=h_sb[r - r0:r - r0 + span, :])
                    r += span

            def do_allgather(qq):
                nc.gpsimd.collective_compute(
                    "AllGather", mybir.AluOpType.bypass,
                    replica_groups=[list(range(NCORES))],
                    ins=[h1_in[qq][:]], outs=[h1_tab[qq][:]])

            ntile_l = -(-nloc // P)
            ag_after_tile = [-(-((q + 1) * qsz) // P) - 1 for q in range(NQ)]

            # ---- one aggregation layer slab ----
            def agg_layer_slab(layer, s, qsel=None):
                    wlo = s * SLAB
                    wid = min(SLAB, nloc - wlo)
                    acc = pspool.tile([P, SLAB], f32, tag="agg_ps")
                    if layer == 1:
                        nc.vector.memset(acc[:], 0.0)
                    for qq in ([qsel] if qsel is not None else range(NQ)):
                        k0, k1 = group_of[(s, qq)]
                        kn = k1 - k0
                        if layer == 0:
                            gt = gpool.tile([P, kn, P], f16, tag="gt")
                            nc.sync.dma_start(
                                out=gt[:], in_=g1[:, k0:k1, :])
                        else:
                            ist = gpool.tile([P, kn * P // 16], mybir.dt.int16,
                                             tag="iststage")
                            nc.sync.dma_start(
                                out=ist[:],
                                in_=idxs[:, k0 * P // 16: k1 * P // 16])
                            gt = gpool.tile([P, kn, P], f16, tag="gt")
                            ni = kn * P
                            nc.gpsimd.dma_gather(
                                gt[:], h1_tab[qq][:], ist[:],
                                ni, ni, P, single_packet=False,
                                queue_num=(s + qq) % 4)
                        # P builds + matmuls
                        k = k0
                        while k < k1:
                            kind = prog[k][2]
                            if kind in ("wide", "ovfl"):
                                if layer == 0:
                                    last = (qq == NQ - 1 and k == k1 - 1)
                                    first = (kind == "wide")
                                else:
                                    last = (k == k1 - 1)
                                    first = False
                                pm = ppool.tile([P, SLAB], f16, tag="pwide")
                                nc.vector.tensor_scalar(
                                    out=pm[:], in0=iota_w[:],
                                    scalar1=off_sb[:, k:k + 1],
                                    scalar2=w_sb[:, k:k + 1],
                                    op0=mybir.AluOpType.is_equal,
                                    op1=mybir.AluOpType.mult)
                                nc.tensor.matmul(
                                    out=acc[:], lhsT=gt[:, k - k0, :], rhs=pm[:],
                                    start=first, stop=last)
                                k += 1
                            else:
                                nb = 1
                                while (nb < PBATCH and k + nb < k1
                                       and prog[k + nb][2] == "norm"):
                                    nb += 1
                                pm = ppool.tile([P, nb, WIN], f16,
                                                tag=f"pn{nb}")
                                bco = bass.AP(
                                    off16.tensor, off16[:, k:k + nb].offset,
                                    [list(off16[:, k:k + nb].ap[0]),
                                     list(off16[:, k:k + nb].ap[1]),
                                     [0, WIN]])
                                bcw = bass.AP(
                                    w16.tensor, w16[:, k:k + nb].offset,
                                    [list(w16[:, k:k + nb].ap[0]),
                                     list(w16[:, k:k + nb].ap[1]),
                                     [0, WIN]])
                                nc.vector.tensor_tensor(
                                    out=pm[:], in0=iota_rep[:, :nb, :],
                                    in1=bco, op=mybir.AluOpType.is_equal)
                                nc.vector.tensor_tensor(
                                    out=pm[:], in0=pm[:], in1=bcw,
                                    op=mybir.AluOpType.mult)
                                for j in range(nb):
                                    base = prog[k + j][3]
                                    if layer == 0:
                                        last = (qq == NQ - 1 and k + j == k1 - 1)
                                    else:
                                        last = (k + j == k1 - 1)
                                    nc.tensor.matmul(
                                        out=acc[:, base:base + WIN],
                                        lhsT=gt[:, k + j - k0, :],
                                        rhs=pm[:, j, :],
                                        start=False, stop=last)
                                k += nb
                    if layer == 0:
                        nc.vector.tensor_copy(out=st_sb[:, wlo:wlo + wid],
                                              in_=acc[:, :wid])
                        return None
                    ev = epool.tile([P, SLAB], f16, tag="l2ev")
                    nc.vector.tensor_copy(out=ev[:, :wid], in_=acc[:, :wid])

                    def epilogue(s=s, qsel=qsel, ev=ev, wid=wid):
                        t0 = (s * SLAB) // P
                        for tt in range(t0, min(t0 + SLAB // P, ntile)):
                            c0 = tt * P - s * SLAB
                            m = min(P, nloc - tt * P)
                            ops = psg2.tile([P, 4], f32, tag="gemm2_ps")
                            nc.tensor.matmul(out=ops[:m, :3],
                                             lhsT=ev[:, c0:c0 + m],
                                             rhs=w2_sb[:], start=True,
                                             stop=(qsel != 0))
                            if qsel == 0:
                                nc.tensor.matmul(out=ops[:m, :3],
                                                 lhsT=ones_sb[:, :m],
                                                 rhs=b2_sb[:], start=False,
                                                 stop=True)
                            nc.vector.tensor_tensor(
                                out=out_acc[:m, tt, :3],
                                in0=out_acc[:m, tt, :3],
                                in1=ops[:m, :3], op=mybir.AluOpType.add)
                    return epilogue

            # ====== layer 1 + layer 2 interleaved emission ======
            # L2 (q,s) groups are emitted between later L1 slabs, as soon as
            # their quarter's AllGather is in the gpsimd stream, keeping every
            # engine's in-order stream dependency-ready.
            l2_queue = [(qq, s) for qq in range(NQ) for s in range(nslab)]
            l2_pos = 0
            pend_ep = []

            def emit_l2(qq2, s2):
                ep = agg_layer_slab(1, s2, qsel=qq2)
                pend_ep.append(ep)
                if len(pend_ep) > 2:
                    pend_ep.pop(0)()
            ag_emitted = 0
            done_tiles = 0
            for s in range(nslab):
                agg_layer_slab(0, s)
                cover = min(ntile_l, ((s + 1) * SLAB) // P) if s < nslab - 1 \
                    else ntile_l
                while done_tiles < cover:
                    gemm1_tile(done_tiles)
                    while ag_emitted < NQ and ag_after_tile[ag_emitted] == done_tiles:
                        do_allgather(ag_emitted)
                        ag_emitted += 1
                    done_tiles += 1
                # pace L2 groups whose table is already in flight
                budget = 3
                while (budget > 0 and l2_pos < len(l2_queue)
                       and l2_queue[l2_pos][0] < ag_emitted):
                    qq2, s2 = l2_queue[l2_pos]
                    emit_l2(qq2, s2)
                    l2_pos += 1
                    budget -= 1
            assert done_tiles == ntile_l and ag_emitted == NQ
            while l2_pos < len(l2_queue):
                qq2, s2 = l2_queue[l2_pos]
                emit_l2(qq2, s2)
                l2_pos += 1
            for ep in pend_ep:
                ep()

            for t in range(ntile):
                r0 = t * P
                m = min(P, nloc - r0)
                nc.sync.dma_start(out=out_t[r0:r0 + m, :],
                                  in_=out_acc[:m, t, :3])

    nc.compile()
    return nc


# ---------------------------------------------------------------------------
# entry point
# ---------------------------------------------------------------------------

def _wrap_idx(vals):
    """int16 gather index layout: slot i -> [i%16, i//16], tiled to 128 rows,
    built per 16-slot column group (layout wraps within each dma_gather call,
    which always covers a whole number of 16-slot columns)."""
    v = vals.reshape(-1, 16).T  # [16, n/16]
    return np.tile(v, (8, 1)).astype(np.int16)


def kernel(x, W1, b1, W2, b2, edge_index, _profile=None):
    import ml_dtypes
    bf16 = ml_dtypes.bfloat16
    x = np.asarray(x, np.float32)
    x16 = x.astype(bf16)
    meta = host_prep(x, edge_index)
    nloc, nch = meta["nloc"], meta["nch"]

    from concourse.bass_utils import run_bass_kernel_spmd
    nc = build_program(meta)

    in_maps = []
    for c in range(NCORES):
        pc = meta["per_core"][c]
        g1 = np.ascontiguousarray(
            x16[pc["slots_src"].reshape(nch, P)].transpose(1, 0, 2))
        in_maps.append(dict(
            g1=g1,
            idxs=_wrap_idx(pc["a_idx"]),
            dstoff=np.ascontiguousarray(pc["a_off"].T),
            dprod=np.ascontiguousarray(pc["a_dpr"].T),
            w1=np.asarray(W1, np.float32).astype(bf16),
            b1=np.asarray(b1, np.float32).reshape(1, P).astype(bf16),
            w2=np.asarray(W2, np.float32).astype(bf16),
            b2=np.asarray(b2, np.float32).reshape(1, 3).astype(bf16),
        ))

    res = run_bass_kernel_spmd(nc, in_maps, list(range(NCORES)),
                               trace=_profile is not None)
    if _profile is not None:
        _profile["exec_time_ns"] = res.exec_time_ns
    out = np.concatenate([res.results[c]["out"] for c in range(NCORES)], axis=0)
    return out.astype(np.float32)



# revision 34
# speedup vs baseline: 1.1317x; 1.0146x over previous
"""Two-layer GCN (PyG GCNConv x2 + leaky_relu(0.2)) on 8 trn2 NeuronCores.

Distribution strategy (dst-sharded graph parallel):
  - Nodes split 8 ways by dst; core c owns dsts [c*NLOC, (c+1)*NLOC).
  - Self-loops appended as edges; full symmetric norm folded into per-edge
    weights w_e = rsqrt((deg[src]+1)*(deg[dst]+1)), computed ON DEVICE from
    integer degree products (uploaded as exact f32 ints).
  - Aggregation = one-hot matmuls on TensorE: per 128-edge chunk,
    lhsT = messages G [128e x 128f] (stationary), rhs = P [128e x 64d]
    (P[e,j] = w_e * (dstoff_e == j), built on VectorE by iota compare),
    accumulating s^T [128f x 512d] PSUM slabs.
  - Chunk dst-windows use SHARED bases (pooled quantiles) so PSUM offsets
    are program literals valid for every core; per-core leftovers go to
    shared full-width overflow chunks.
  - Layer-1 messages (x[src]) are host-prelaid in chunk-slot order and
    streamed sequentially (HWDGE).  Layer-2 messages (h1[src]) are gathered
    on device (gpsimd dma_gather) from 4 AllGather'd h1 tables of
    NLOC*2 = 25000 rows each (int16-index-safe, no offset views).
  - GEMMs per 128-node tile: h1 = Prelu_{0.2}(s1 @ W1 + b1) (bias via rank-1
    matmul into the same PSUM group), out = s2 @ W2 + b2.
"""

import os
import sys

import numpy as np

sys.path.insert(0, "/opt/trn_rl_repo")

P = 128
NCORES = 8
SLAB = 512       # dsts per PSUM slab (one 2KB fp32 bank)
WIN = 64         # narrow-chunk P width
NQ = 4           # gather-table quarters (int16 index range)
STEP_FRAC = 1.0 # target per-core edges per scheduled chunk / 128
PBATCH = 8       # narrow chunks per batched DVE P-build op


# ---------------------------------------------------------------------------
# host-side structure prep
# ---------------------------------------------------------------------------

def _chunk_schedule(dl_pooled, n_max):
    """Shared window bases for one (slab, q) group from pooled local dsts.

    Returns monotone bases (step clamped to <= WIN) for K scheduled chunks.
    """
    if n_max == 0:
        return np.zeros(1, np.int64)
    step = max(1, int(P * STEP_FRAC))
    K = max(1, -(-n_max // step))
    npool = len(dl_pooled)
    bases = []
    prev = 0
    for k in range(K):
        b = int(dl_pooled[min(int(k * npool / K), npool - 1)]) if npool else 0
        b = max(prev if k else 0, b - 8)           # small low-side margin
        if k:
            b = min(b, prev + WIN)                 # reachability clamp
            b = max(b, prev)                       # monotone
        b = min(b, SLAB - WIN)
        bases.append(b)
        prev = b
    return np.asarray(bases, np.int64)


def _fill_core(dl, bases):
    """Greedy fill of one core's sorted dsts into scheduled windows.

    Returns list of (i0, i1, base) per scheduled chunk + leftover index list.
    """
    n = len(dl)
    out = []
    leftover = []
    ptr = 0
    for b in bases:
        lo = ptr + int(np.searchsorted(dl[ptr:], b))
        if lo > ptr:
            leftover.extend(range(ptr, lo))        # below-window stragglers
        hi = lo + int(np.searchsorted(dl[lo:], b + WIN))
        j = min(lo + P, hi)
        out.append((lo, j, int(b)))
        ptr = j
    leftover.extend(range(ptr, n))
    return out, leftover


def host_prep(x, edge_index):
    n_nodes = x.shape[0]
    nloc = n_nodes // NCORES
    qsz = nloc // NQ
    nslab = -(-nloc // SLAB)
    src = np.asarray(edge_index[0], np.int64)
    dst = np.asarray(edge_index[1], np.int64)

    deg = np.bincount(dst, minlength=n_nodes).astype(np.int64)
    srcA = np.concatenate([src, np.arange(n_nodes, dtype=np.int64)])
    dstA = np.concatenate([dst, np.arange(n_nodes, dtype=np.int64)])
    degp = deg + 1
    degprod = (degp[srcA] * degp[dstA]).astype(np.float32)  # exact (< 2^24)

    core = dstA // nloc
    dloc = dstA % nloc
    slab = dloc // SLAB
    dsl = dloc - slab * SLAB
    q = (srcA % nloc) // qsz
    idxval = (qsz * (srcA // nloc) + (srcA % qsz)).astype(np.int32)

    order = np.lexsort((dsl, q, slab, core))
    srcA = srcA[order]; dsl = dsl[order]; slab = slab[order]
    q = q[order]; core = core[order]
    idxval = idxval[order]; degprod = degprod[order]

    key = (core * nslab + slab) * NQ + q
    starts = np.searchsorted(key, np.arange(NCORES * nslab * NQ + 1))

    def grp(c, s, qq):
        g = (c * nslab + s) * NQ + qq
        return int(starts[g]), int(starts[g + 1])

    # --- shared schedule per (slab, q): bases + total chunk count ---------
    sched = {}
    for s in range(nslab):
        for qq in range(NQ):
            segs = [grp(c, s, qq) for c in range(NCORES)]
            pooled = np.sort(np.concatenate([dsl[a:b] for a, b in segs]))
            n_max = max(b - a for a, b in segs)
            bases = _chunk_schedule(pooled, n_max)
            fills = []
            ov_max = 0
            for c in range(NCORES):
                a, b = segs[c]
                f, lo = _fill_core(dsl[a:b], bases)
                fills.append((a, f, lo))
                ov_max = max(ov_max, -(-len(lo) // P))
            sched[(s, qq)] = (bases, fills, ov_max)

    # chunk meta in program order: (s, qq, kind, base) ; kind: 'wide' first
    # chunk of each slab (width SLAB, start=True), 'norm' width WIN,
    # 'ovfl' width SLAB.
    prog = []
    for s in range(nslab):
        for qq in range(NQ):
            bases, _, ov_max = sched[(s, qq)]
            for k in range(len(bases)):
                if qq == 0 and k == 0:
                    prog.append((s, qq, "wide", 0))
                else:
                    prog.append((s, qq, "norm", int(bases[k])))
            for _ in range(ov_max):
                prog.append((s, qq, "ovfl", 0))
    nch = len(prog)

    # --- per-core slot arrays --------------------------------------------
    per_core = []
    for c in range(NCORES):
        slots_src = np.zeros(nch * P, np.int64)
        a_off = np.full((nch, P), -1.0, np.float32)
        a_dpr = np.ones((nch, P), np.float32)
        a_idx = np.zeros(nch * P, np.int32)
        ci = 0
        for s in range(nslab):
            for qq in range(NQ):
                bases, fills, ov_max = sched[(s, qq)]
                a, f, lo = fills[c]
                for k in range(len(bases)):
                    i0, i1, b = f[k]
                    m = i1 - i0
                    if m > 0:
                        sl = slice(ci * P, ci * P + m)
                        rows = slice(a + i0, a + i1)
                        slots_src[sl] = srcA[rows]
                        a_idx[sl] = idxval[rows]
                        a_dpr[ci, :m] = degprod[rows]
                        if qq == 0 and k == 0:
                            a_off[ci, :m] = dsl[rows]
                        else:
                            a_off[ci, :m] = dsl[rows] - b
                    ci += 1
                for o in range(ov_max):
                    idxs = lo[o * P:(o + 1) * P]
                    m = len(idxs)
                    if m > 0:
                        rows = a + np.asarray(idxs, np.int64)
                        sl = slice(ci * P, ci * P + m)
                        slots_src[sl] = srcA[rows]
                        a_idx[sl] = idxval[rows]
                        a_dpr[ci, :m] = degprod[rows]
                        a_off[ci, :m] = dsl[rows]
                    ci += 1
        assert ci == nch
        per_core.append(dict(slots_src=slots_src, a_off=a_off, a_dpr=a_dpr,
                             a_idx=a_idx))

    # sanity: every edge placed exactly once
    placed = sum((pc["a_off"] >= 0).sum() for pc in per_core)
    assert placed == len(srcA), (placed, len(srcA))

    return dict(n_nodes=n_nodes, nloc=nloc, qsz=qsz, nslab=nslab, nch=nch,
                prog=prog, per_core=per_core)


# ---------------------------------------------------------------------------
# device program
# ---------------------------------------------------------------------------

def build_program(meta):
    import concourse.bacc as bacc
    import concourse.bass as bass
    import concourse.tile as tile
    from concourse import mybir

    nloc, qsz, nslab, nch = meta["nloc"], meta["qsz"], meta["nslab"], meta["nch"]
    prog = meta["prog"]
    tabrows = qsz * NCORES
    f32 = mybir.dt.float32
    f16 = mybir.dt.bfloat16

    nc = bacc.Bacc("TRN2", target_bir_lowering=False, debug=False,
                   num_devices=NCORES, num_swdge_queues=4)

    g1 = nc.dram_tensor("g1", [P, nch, P], f16, kind="ExternalInput")
    idxs = nc.dram_tensor("idxs", [P, nch * P // 16], mybir.dt.int16,
                          kind="ExternalInput")
    dstoff = nc.dram_tensor("dstoff", [P, nch], f32, kind="ExternalInput")
    dprod = nc.dram_tensor("dprod", [P, nch], f32, kind="ExternalInput")
    w1_t = nc.dram_tensor("w1", [P, P], f16, kind="ExternalInput")
    b1_t = nc.dram_tensor("b1", [1, P], f16, kind="ExternalInput")
    w2_t = nc.dram_tensor("w2", [P, 3], f16, kind="ExternalInput")
    b2_t = nc.dram_tensor("b2", [1, 3], f16, kind="ExternalInput")
    out_t = nc.dram_tensor("out", [nloc, 3], f32, kind="ExternalOutput")

    h1_in = [nc.dram_tensor(f"h1_in{qq}", [qsz, P], f16) for qq in range(NQ)]
    h1_tab = [nc.dram_tensor(f"h1_tab{qq}", [tabrows, P], f16,
                             addr_space="Shared") for qq in range(NQ)]

    # chunk ranges per (s, qq): [k0, k1) in program order + gather call list
    group_of = {}
    pos = 0
    for s in range(nslab):
        for qq in range(NQ):
            k0 = pos
            while pos < nch and prog[pos][0] == s and prog[pos][1] == qq:
                pos += 1
            group_of[(s, qq)] = (k0, pos)
    assert pos == nch

    ntile = -(-nloc // P)

    with tile.TileContext(nc) as tc:
        with tc.tile_pool(name="const", bufs=1) as cpool, \
             tc.tile_pool(name="stsb", bufs=1) as spool, \
             tc.tile_pool(name="gbuf", bufs=6) as gpool, \
             tc.tile_pool(name="pbuf", bufs=10) as ppool, \
             tc.tile_pool(name="evbuf", bufs=6) as epool, \
             tc.tile_pool(name="psum", bufs=5, space="PSUM") as pspool, \
             tc.tile_pool(name="psg", bufs=2, space="PSUM") as psg, \
             tc.tile_pool(name="psg2", bufs=1, space="PSUM") as psg2:

            # ---- constants / structure loads ----
            off_sb = cpool.tile([P, nch], f32)
            nc.sync.dma_start(out=off_sb[:], in_=dstoff[:])
            dpr_sb = cpool.tile([P, nch], f32)
            nc.sync.dma_start(out=dpr_sb[:], in_=dprod[:])
            rc_sb = cpool.tile([P, nch], f32)
            nc.vector.reciprocal(out=rc_sb[:], in_=dpr_sb[:])
            w_sb = cpool.tile([P, nch], f32)
            nc.scalar.activation(out=w_sb[:], in_=rc_sb[:],
                                 func=mybir.ActivationFunctionType.Sqrt)
            off16 = cpool.tile([P, nch], f16)
            nc.vector.tensor_copy(out=off16[:], in_=off_sb[:])
            w16 = cpool.tile([P, nch], f16)
            nc.vector.tensor_copy(out=w16[:], in_=w_sb[:])

            iota_w = cpool.tile([P, SLAB], f32)
            nc.gpsimd.iota(iota_w[:], [[1, SLAB]], base=0, channel_multiplier=0,
                           allow_small_or_imprecise_dtypes=True)
            iota_rep = cpool.tile([P, PBATCH, WIN], f16)
            for jj in range(PBATCH):
                nc.vector.tensor_copy(out=iota_rep[:, jj, :],
                                      in_=iota_w[:, :WIN])

            w1_sb = cpool.tile([P, P], f16)
            nc.sync.dma_start(out=w1_sb[:], in_=w1_t[:])
            b1_sb = cpool.tile([1, P], f16)
            nc.sync.dma_start(out=b1_sb[:], in_=b1_t[:])
            w2_sb = cpool.tile([P, 3], f16)
            nc.sync.dma_start(out=w2_sb[:], in_=w2_t[:])
            b2_sb = cpool.tile([1, 3], f16)
            nc.sync.dma_start(out=b2_sb[:], in_=b2_t[:])
            ones_sb = cpool.tile([1, P], f16)
            nc.vector.memset(ones_sb[:], 1.0)
            alpha_sb = cpool.tile([P, 1], f32)
            nc.vector.memset(alpha_sb[:], 0.2)

            st_sb = spool.tile([P, nloc], f16, tag="stT")  # s1T (layer 1)
            out_acc = spool.tile([P, ntile, 4], f32, tag="outacc")
            nc.vector.memset(out_acc[:], 0.0)

            def gemm1_tile(t):
                r0 = t * P
                m = min(P, nloc - r0)
                hps = psg.tile([P, P], f32, tag="gemm_ps")
                nc.tensor.matmul(out=hps[:m, :], lhsT=st_sb[:, r0:r0 + m],
                                 rhs=w1_sb[:], start=True, stop=False)
                nc.tensor.matmul(out=hps[:m, :], lhsT=ones_sb[:, :m],
                                 rhs=b1_sb[:], start=False, stop=True)
                h_sb = epool.tile([P, P], f16, tag="h1t")
                nc.scalar.activation(out=h_sb[:m, :], in_=hps[:m, :],
                                     func=mybir.ActivationFunctionType.Prelu,
                                     alpha=alpha_sb[:m, 0:1])
                r = r0
                while r < r0 + m:
                    qq = r // qsz
                    rq = r - qq * qsz
                    span = min(r0 + m - r, qsz - rq)
                    nc.sync.dma_start(
                        out=h1_in[qq][rq:rq + span, :],
                        in_=h_sb[r - r0:r - r0 + span, :])
                    r += span

            def do_allgather(qq):
                nc.gpsimd.collective_compute(
                    "AllGather", mybir.AluOpType.bypass,
                    replica_groups=[list(range(NCORES))],
                    ins=[h1_in[qq][:]], outs=[h1_tab[qq][:]])

            ntile_l = -(-nloc // P)
            ag_after_tile = [-(-((q + 1) * qsz) // P) - 1 for q in range(NQ)]

            # ---- one aggregation layer slab ----
            def agg_layer_slab(layer, s, qsel=None):
                    wlo = s * SLAB
                    wid = min(SLAB, nloc - wlo)
                    acc = pspool.tile([P, SLAB], f32, tag="agg_ps")
                    if layer == 1:
                        nc.vector.memset(acc[:], 0.0)
                    for qq in ([qsel] if qsel is not None else range(NQ)):
                        k0, k1 = group_of[(s, qq)]
                        kn = k1 - k0
                        if layer == 0:
                            gt = gpool.tile([P, kn, P], f16, tag="gt")
                            nc.sync.dma_start(
                                out=gt[:], in_=g1[:, k0:k1, :])
                        else:
                            ist = gpool.tile([P, kn * P // 16], mybir.dt.int16,
                                             tag="iststage")
                            nc.sync.dma_start(
                                out=ist[:],
                                in_=idxs[:, k0 * P // 16: k1 * P // 16])
                            gt = gpool.tile([P, kn, P], f16, tag="gt")
                            ni = kn * P
                            nc.gpsimd.dma_gather(
                                gt[:], h1_tab[qq][:], ist[:],
                                ni, ni, P, single_packet=False,
                                queue_num=(s + qq) % 4)
                        # P builds + matmuls
                        k = k0
                        while k < k1:
                            kind = prog[k][2]
                            if kind in ("wide", "ovfl"):
                                if layer == 0:
                                    last = (qq == NQ - 1 and k == k1 - 1)
                                    first = (kind == "wide")
                                else:
                                    last = (k == k1 - 1)
                                    first = False
                                pm = ppool.tile([P, SLAB], f16, tag="pwide")
                                nc.vector.tensor_scalar(
                                    out=pm[:], in0=iota_w[:],
                                    scalar1=off_sb[:, k:k + 1],
                                    scalar2=w_sb[:, k:k + 1],
                                    op0=mybir.AluOpType.is_equal,
                                    op1=mybir.AluOpType.mult)
                                nc.tensor.matmul(
                                    out=acc[:], lhsT=gt[:, k - k0, :], rhs=pm[:],
                                    start=first, stop=last)
                                k += 1
                            else:
                                nb = 1
                                while (nb < PBATCH and k + nb < k1
                                       and prog[k + nb][2] == "norm"):
                                    nb += 1
                                pm = ppool.tile([P, nb, WIN], f16,
                                                tag=f"pn{nb}")
                                bco = bass.AP(
                                    off16.tensor, off16[:, k:k + nb].offset,
                                    [list(off16[:, k:k + nb].ap[0]),
                                     list(off16[:, k:k + nb].ap[1]),
                                     [0, WIN]])
                                bcw = bass.AP(
                                    w16.tensor, w16[:, k:k + nb].offset,
                                    [list(w16[:, k:k + nb].ap[0]),
                                     list(w16[:, k:k + nb].ap[1]),
                                     [0, WIN]])
                                nc.vector.tensor_tensor(
                                    out=pm[:], in0=iota_rep[:, :nb, :],
                                    in1=bco, op=mybir.AluOpType.is_equal)
                                nc.vector.tensor_tensor(
                                    out=pm[:], in0=pm[:], in1=bcw,
                                    op=mybir.AluOpType.mult)
                                for j in range(nb):
                                    base = prog[k + j][3]
                                    if layer == 0:
                                        last = (qq == NQ - 1 and k + j == k1 - 1)
                                    else:
                                        last = (k + j == k1 - 1)
                                    nc.tensor.matmul(
                                        out=acc[:, base:base + WIN],
                                        lhsT=gt[:, k + j - k0, :],
                                        rhs=pm[:, j, :],
                                        start=False, stop=last)
                                k += nb
                    if layer == 0:
                        nc.vector.tensor_copy(out=st_sb[:, wlo:wlo + wid],
                                              in_=acc[:, :wid])
                        return None
                    ev = epool.tile([P, SLAB], f16, tag="l2ev")
                    nc.vector.tensor_copy(out=ev[:, :wid], in_=acc[:, :wid])

                    def epilogue(s=s, qsel=qsel, ev=ev, wid=wid):
                        t0 = (s * SLAB) // P
                        for tt in range(t0, min(t0 + SLAB // P, ntile)):
                            c0 = tt * P - s * SLAB
                            m = min(P, nloc - tt * P)
                            ops = psg2.tile([P, 4], f32, tag="gemm2_ps")
                            nc.tensor.matmul(out=ops[:m, :3],
                                             lhsT=ev[:, c0:c0 + m],
                                             rhs=w2_sb[:], start=True,
                                             stop=(qsel != 0))
                            if qsel == 0:
                                nc.tensor.matmul(out=ops[:m, :3],
                                                 lhsT=ones_sb[:, :m],
                                                 rhs=b2_sb[:], start=False,
                                                 stop=True)
                            nc.vector.tensor_tensor(
                                out=out_acc[:m, tt, :3],
                                in0=out_acc[:m, tt, :3],
                                in1=ops[:m, :3], op=mybir.AluOpType.add)
                    return epilogue

            # ====== layer 1 + layer 2 interleaved emission ======
            # L2 (q,s) groups are emitted between later L1 slabs, as soon as
            # their quarter's AllGather is in the gpsimd stream, keeping every
            # engine's in-order stream dependency-ready.
            l2_queue = [(qq, s) for qq in range(NQ) for s in range(nslab)]
            l2_pos = 0
            pend_ep = []

            def emit_l2(qq2, s2):
                ep = agg_layer_slab(1, s2, qsel=qq2)
                pend_ep.append(ep)
                if len(pend_ep) > 2:
                    pend_ep.pop(0)()
            ag_emitted = 0
            done_tiles = 0
            for s in range(nslab):
                agg_layer_slab(0, s)
                cover = min(ntile_l, ((s + 1) * SLAB) // P) if s < nslab - 1 \
                    else ntile_l
                while done_tiles < cover:
                    gemm1_tile(done_tiles)
                    while ag_emitted < NQ and ag_after_tile[ag_emitted] == done_tiles:
                        do_allgather(ag_emitted)
                        ag_emitted += 1
                    done_tiles += 1
                # pace L2 groups whose table is already in flight
                budget = 3
                while (budget > 0 and l2_pos < len(l2_queue)
                       and l2_queue[l2_pos][0] < ag_emitted):
                    qq2, s2 = l2_queue[l2_pos]
                    emit_l2(qq2, s2)
                    l2_pos += 1
                    budget -= 1
            assert done_tiles == ntile_l and ag_emitted == NQ
            while l2_pos < len(l2_queue):
                qq2, s2 = l2_queue[l2_pos]
                emit_l2(qq2, s2)
                l2_pos += 1
            for ep in pend_ep:
                ep()

            for t in range(ntile):
                r0 = t * P
                m = min(P, nloc - r0)
                nc.sync.dma_start(out=out_t[r0:r0 + m, :],
                                  in_=out_acc[:m, t, :3])

    nc.compile()
    return nc


# ---------------------------------------------------------------------------
# entry point
# ---------------------------------------------------------------------------

def _wrap_idx(vals):
    """int16 gather index layout: slot i -> [i%16, i//16], tiled to 128 rows,
    built per 16-slot column group (layout wraps within each dma_gather call,
    which always covers a whole number of 16-slot columns)."""
    v = vals.reshape(-1, 16).T  # [16, n/16]
    return np.tile(v, (8, 1)).astype(np.int16)


def kernel(x, W1, b1, W2, b2, edge_index, _profile=None):
    import ml_dtypes
    bf16 = ml_dtypes.bfloat16
    x = np.asarray(x, np.float32)
    x16 = x.astype(bf16)
    meta = host_prep(x, edge_index)
    nloc, nch = meta["nloc"], meta["nch"]

    from concourse.bass_utils import run_bass_kernel_spmd
    nc = build_program(meta)

    in_maps = []
    for c in range(NCORES):
        pc = meta["per_core"][c]
        g1 = np.ascontiguousarray(
            x16[pc["slots_src"].reshape(nch, P)].transpose(1, 0, 2))
        in_maps.append(dict(
            g1=g1,
            idxs=_wrap_idx(pc["a_idx"]),
            dstoff=np.ascontiguousarray(pc["a_off"].T),
            dprod=np.ascontiguousarray(pc["a_dpr"].T),
            w1=np.asarray(W1, np.float32).astype(bf16),
            b1=np.asarray(b1, np.float32).reshape(1, P).astype(bf16),
            w2=np.asarray(W2, np.float32).astype(bf16),
            b2=np.asarray(b2, np.float32).reshape(1, 3).astype(bf16),
        ))

    res = run_bass_kernel_spmd(nc, in_maps, list(range(NCORES)),
                               trace=_profile is not None)
    if _profile is not None:
        _profile["exec_time_ns"] = res.exec_time_ns
    out = np.concatenate([res.results[c]["out"] for c in range(NCORES)], axis=0)
    return out.astype(np.float32)



# revision 35
# speedup vs baseline: 1.2676x; 1.1201x over previous
"""Two-layer GCN (PyG GCNConv x2 + leaky_relu(0.2)) on 8 trn2 NeuronCores.

Distribution strategy (dst-sharded graph parallel):
  - Nodes split 8 ways by dst; core c owns dsts [c*NLOC, (c+1)*NLOC).
  - Self-loops appended as edges; full symmetric norm folded into per-edge
    weights w_e = rsqrt((deg[src]+1)*(deg[dst]+1)), computed ON DEVICE from
    integer degree products (uploaded as exact f32 ints).
  - Aggregation = one-hot matmuls on TensorE: per 128-edge chunk,
    lhsT = messages G [128e x 128f] (stationary), rhs = P [128e x 64d]
    (P[e,j] = w_e * (dstoff_e == j), built on VectorE by iota compare),
    accumulating s^T [128f x 512d] PSUM slabs.
  - Chunk dst-windows use SHARED bases (pooled quantiles) so PSUM offsets
    are program literals valid for every core; per-core leftovers go to
    shared full-width overflow chunks.
  - Layer-1 messages (x[src]) are host-prelaid in chunk-slot order and
    streamed sequentially (HWDGE).  Layer-2 messages (h1[src]) are gathered
    on device (gpsimd dma_gather) from 4 AllGather'd h1 tables of
    NLOC*2 = 25000 rows each (int16-index-safe, no offset views).
  - GEMMs per 128-node tile: h1 = Prelu_{0.2}(s1 @ W1 + b1) (bias via rank-1
    matmul into the same PSUM group), out = s2 @ W2 + b2.
"""

import os
import sys

import numpy as np

sys.path.insert(0, "/opt/trn_rl_repo")

P = 128
NCORES = 8
SLAB = 512       # dsts per PSUM slab (one 2KB fp32 bank)
WIN = 64         # narrow-chunk P width
NQ = 4           # gather-table quarters (int16 index range)
STEP_FRAC = 1.0 # target per-core edges per scheduled chunk / 128
PBATCH = 16      # narrow chunks per batched DVE P-build op


# ---------------------------------------------------------------------------
# host-side structure prep
# ---------------------------------------------------------------------------

def _chunk_schedule(dl_pooled, n_max):
    """Shared window bases for one (slab, q) group from pooled local dsts.

    Returns monotone bases (step clamped to <= WIN) for K scheduled chunks.
    """
    if n_max == 0:
        return np.zeros(1, np.int64)
    step = max(1, int(P * STEP_FRAC))
    K = max(1, -(-n_max // step))
    npool = len(dl_pooled)
    bases = []
    prev = 0
    for k in range(K):
        b = int(dl_pooled[min(int(k * npool / K), npool - 1)]) if npool else 0
        b = max(prev if k else 0, b - 8)           # small low-side margin
        if k:
            b = min(b, prev + WIN)                 # reachability clamp
            b = max(b, prev)                       # monotone
        b = min(b, SLAB - WIN)
        bases.append(b)
        prev = b
    return np.asarray(bases, np.int64)


def _fill_core(dl, bases):
    """Greedy fill of one core's sorted dsts into scheduled windows.

    Returns list of (i0, i1, base) per scheduled chunk + leftover index list.
    """
    n = len(dl)
    out = []
    leftover = []
    ptr = 0
    for b in bases:
        lo = ptr + int(np.searchsorted(dl[ptr:], b))
        if lo > ptr:
            leftover.extend(range(ptr, lo))        # below-window stragglers
        hi = lo + int(np.searchsorted(dl[lo:], b + WIN))
        j = min(lo + P, hi)
        out.append((lo, j, int(b)))
        ptr = j
    leftover.extend(range(ptr, n))
    return out, leftover


def host_prep(x, edge_index):
    n_nodes = x.shape[0]
    nloc = n_nodes // NCORES
    qsz = nloc // NQ
    nslab = -(-nloc // SLAB)
    src = np.asarray(edge_index[0], np.int64)
    dst = np.asarray(edge_index[1], np.int64)

    deg = np.bincount(dst, minlength=n_nodes).astype(np.int64)
    srcA = np.concatenate([src, np.arange(n_nodes, dtype=np.int64)])
    dstA = np.concatenate([dst, np.arange(n_nodes, dtype=np.int64)])
    degp = deg + 1
    degprod = (degp[srcA] * degp[dstA]).astype(np.float32)  # exact (< 2^24)

    core = dstA // nloc
    dloc = dstA % nloc
    slab = dloc // SLAB
    dsl = dloc - slab * SLAB
    q = (srcA % nloc) // qsz
    idxval = (qsz * (srcA // nloc) + (srcA % qsz)).astype(np.int32)

    order = np.lexsort((dsl, q, slab, core))
    srcA = srcA[order]; dsl = dsl[order]; slab = slab[order]
    q = q[order]; core = core[order]
    idxval = idxval[order]; degprod = degprod[order]

    key = (core * nslab + slab) * NQ + q
    starts = np.searchsorted(key, np.arange(NCORES * nslab * NQ + 1))

    def grp(c, s, qq):
        g = (c * nslab + s) * NQ + qq
        return int(starts[g]), int(starts[g + 1])

    # --- shared schedule per (slab, q): bases + total chunk count ---------
    sched = {}
    for s in range(nslab):
        for qq in range(NQ):
            segs = [grp(c, s, qq) for c in range(NCORES)]
            pooled = np.sort(np.concatenate([dsl[a:b] for a, b in segs]))
            n_max = max(b - a for a, b in segs)
            bases = _chunk_schedule(pooled, n_max)
            fills = []
            ov_max = 0
            for c in range(NCORES):
                a, b = segs[c]
                f, lo = _fill_core(dsl[a:b], bases)
                fills.append((a, f, lo))
                ov_max = max(ov_max, -(-len(lo) // P))
            sched[(s, qq)] = (bases, fills, ov_max)

    # chunk meta in program order: (s, qq, kind, base) ; kind: 'wide' first
    # chunk of each slab (width SLAB, start=True), 'norm' width WIN,
    # 'ovfl' width SLAB.
    prog = []
    for s in range(nslab):
        for qq in range(NQ):
            bases, _, ov_max = sched[(s, qq)]
            for k in range(len(bases)):
                if qq == 0 and k == 0:
                    prog.append((s, qq, "wide", 0))
                else:
                    prog.append((s, qq, "norm", int(bases[k])))
            for _ in range(ov_max):
                prog.append((s, qq, "ovfl", 0))
    nch = len(prog)

    # --- per-core slot arrays --------------------------------------------
    per_core = []
    for c in range(NCORES):
        slots_src = np.zeros(nch * P, np.int64)
        a_off = np.full((nch, P), -1.0, np.float32)
        a_dpr = np.ones((nch, P), np.float32)
        a_idx = np.zeros(nch * P, np.int32)
        ci = 0
        for s in range(nslab):
            for qq in range(NQ):
                bases, fills, ov_max = sched[(s, qq)]
                a, f, lo = fills[c]
                for k in range(len(bases)):
                    i0, i1, b = f[k]
                    m = i1 - i0
                    if m > 0:
                        sl = slice(ci * P, ci * P + m)
                        rows = slice(a + i0, a + i1)
                        slots_src[sl] = srcA[rows]
                        a_idx[sl] = idxval[rows]
                        a_dpr[ci, :m] = degprod[rows]
                        if qq == 0 and k == 0:
                            a_off[ci, :m] = dsl[rows]
                        else:
                            a_off[ci, :m] = dsl[rows] - b
                    ci += 1
                for o in range(ov_max):
                    idxs = lo[o * P:(o + 1) * P]
                    m = len(idxs)
                    if m > 0:
                        rows = a + np.asarray(idxs, np.int64)
                        sl = slice(ci * P, ci * P + m)
                        slots_src[sl] = srcA[rows]
                        a_idx[sl] = idxval[rows]
                        a_dpr[ci, :m] = degprod[rows]
                        a_off[ci, :m] = dsl[rows]
                    ci += 1
        assert ci == nch
        per_core.append(dict(slots_src=slots_src, a_off=a_off, a_dpr=a_dpr,
                             a_idx=a_idx))

    # sanity: every edge placed exactly once
    placed = sum((pc["a_off"] >= 0).sum() for pc in per_core)
    assert placed == len(srcA), (placed, len(srcA))

    return dict(n_nodes=n_nodes, nloc=nloc, qsz=qsz, nslab=nslab, nch=nch,
                prog=prog, per_core=per_core)


# ---------------------------------------------------------------------------
# device program
# ---------------------------------------------------------------------------

def build_program(meta):
    import concourse.bacc as bacc
    import concourse.bass as bass
    import concourse.tile as tile
    from concourse import mybir

    nloc, qsz, nslab, nch = meta["nloc"], meta["qsz"], meta["nslab"], meta["nch"]
    prog = meta["prog"]
    tabrows = qsz * NCORES
    f32 = mybir.dt.float32
    f16 = mybir.dt.bfloat16

    nc = bacc.Bacc("TRN2", target_bir_lowering=False, debug=False,
                   num_devices=NCORES, num_swdge_queues=4)

    g1 = nc.dram_tensor("g1", [P, nch, P], f16, kind="ExternalInput")
    idxs = nc.dram_tensor("idxs", [P, nch * P // 16], mybir.dt.int16,
                          kind="ExternalInput")
    dstoff = nc.dram_tensor("dstoff", [P, nch], f32, kind="ExternalInput")
    dprod = nc.dram_tensor("dprod", [P, nch], f32, kind="ExternalInput")
    w1_t = nc.dram_tensor("w1", [P, P], f16, kind="ExternalInput")
    b1_t = nc.dram_tensor("b1", [1, P], f16, kind="ExternalInput")
    w2_t = nc.dram_tensor("w2", [P, 3], f16, kind="ExternalInput")
    b2_t = nc.dram_tensor("b2", [1, 3], f16, kind="ExternalInput")
    out_t = nc.dram_tensor("out", [nloc, 3], f32, kind="ExternalOutput")

    h1_in = [nc.dram_tensor(f"h1_in{qq}", [qsz, P], f16) for qq in range(NQ)]
    h1_tab = [nc.dram_tensor(f"h1_tab{qq}", [tabrows, P], f16,
                             addr_space="Shared") for qq in range(NQ)]

    # chunk ranges per (s, qq): [k0, k1) in program order + gather call list
    group_of = {}
    pos = 0
    for s in range(nslab):
        for qq in range(NQ):
            k0 = pos
            while pos < nch and prog[pos][0] == s and prog[pos][1] == qq:
                pos += 1
            group_of[(s, qq)] = (k0, pos)
    assert pos == nch

    ntile = -(-nloc // P)

    with tile.TileContext(nc) as tc:
        with tc.tile_pool(name="const", bufs=1) as cpool, \
             tc.tile_pool(name="stsb", bufs=1) as spool, \
             tc.tile_pool(name="gbuf", bufs=6) as gpool, \
             tc.tile_pool(name="pbuf", bufs=10) as ppool, \
             tc.tile_pool(name="evbuf", bufs=6) as epool, \
             tc.tile_pool(name="psum", bufs=5, space="PSUM") as pspool, \
             tc.tile_pool(name="psg", bufs=2, space="PSUM") as psg, \
             tc.tile_pool(name="psg2", bufs=1, space="PSUM") as psg2:

            # ---- constants / structure loads ----
            off_sb = cpool.tile([P, nch], f32)
            nc.sync.dma_start(out=off_sb[:], in_=dstoff[:])
            dpr_sb = cpool.tile([P, nch], f32)
            nc.sync.dma_start(out=dpr_sb[:], in_=dprod[:])
            rc_sb = cpool.tile([P, nch], f32)
            nc.vector.reciprocal(out=rc_sb[:], in_=dpr_sb[:])
            w_sb = cpool.tile([P, nch], f32)
            nc.scalar.activation(out=w_sb[:], in_=rc_sb[:],
                                 func=mybir.ActivationFunctionType.Sqrt)
            off16 = cpool.tile([P, nch], f16)
            nc.vector.tensor_copy(out=off16[:], in_=off_sb[:])
            w16 = cpool.tile([P, nch], f16)
            nc.vector.tensor_copy(out=w16[:], in_=w_sb[:])

            iota_w = cpool.tile([P, SLAB], f32)
            nc.gpsimd.iota(iota_w[:], [[1, SLAB]], base=0, channel_multiplier=0,
                           allow_small_or_imprecise_dtypes=True)
            iota_rep = cpool.tile([P, PBATCH, WIN], f16)
            for jj in range(PBATCH):
                nc.vector.tensor_copy(out=iota_rep[:, jj, :],
                                      in_=iota_w[:, :WIN])

            w1_sb = cpool.tile([P, P], f16)
            nc.sync.dma_start(out=w1_sb[:], in_=w1_t[:])
            b1_sb = cpool.tile([1, P], f16)
            nc.sync.dma_start(out=b1_sb[:], in_=b1_t[:])
            w2_sb = cpool.tile([P, 3], f16)
            nc.sync.dma_start(out=w2_sb[:], in_=w2_t[:])
            b2_sb = cpool.tile([1, 3], f16)
            nc.sync.dma_start(out=b2_sb[:], in_=b2_t[:])
            ones_sb = cpool.tile([1, P], f16)
            nc.vector.memset(ones_sb[:], 1.0)
            alpha_sb = cpool.tile([P, 1], f32)
            nc.vector.memset(alpha_sb[:], 0.2)
            alpha1_sb = cpool.tile([P, 1], f32)
            nc.vector.memset(alpha1_sb[:], 1.0)

            st_sb = spool.tile([P, nloc], f16, tag="stT")  # s1T (layer 1)
            out_acc = spool.tile([P, ntile, 4], f32, tag="outacc")
            nc.vector.memset(out_acc[:], 0.0)

            def gemm1_tile(t):
                r0 = t * P
                m = min(P, nloc - r0)
                hps = psg.tile([P, P], f32, tag="gemm_ps")
                nc.tensor.matmul(out=hps[:m, :], lhsT=st_sb[:, r0:r0 + m],
                                 rhs=w1_sb[:], start=True, stop=False)
                nc.tensor.matmul(out=hps[:m, :], lhsT=ones_sb[:, :m],
                                 rhs=b1_sb[:], start=False, stop=True)
                h_sb = epool.tile([P, P], f16, tag="h1t")
                nc.scalar.activation(out=h_sb[:m, :], in_=hps[:m, :],
                                     func=mybir.ActivationFunctionType.Prelu,
                                     alpha=alpha_sb[:m, 0:1])
                r = r0
                while r < r0 + m:
                    qq = r // qsz
                    rq = r - qq * qsz
                    span = min(r0 + m - r, qsz - rq)
                    nc.sync.dma_start(
                        out=h1_in[qq][rq:rq + span, :],
                        in_=h_sb[r - r0:r - r0 + span, :])
                    r += span

            def do_allgather(qq):
                nc.gpsimd.collective_compute(
                    "AllGather", mybir.AluOpType.bypass,
                    replica_groups=[list(range(NCORES))],
                    ins=[h1_in[qq][:]], outs=[h1_tab[qq][:]])

            ntile_l = -(-nloc // P)
            ag_after_tile = [-(-((q + 1) * qsz) // P) - 1 for q in range(NQ)]

            # ---- one aggregation layer slab ----
            def agg_layer_slab(layer, s, qsel=None):
                    wlo = s * SLAB
                    wid = min(SLAB, nloc - wlo)
                    acc = pspool.tile([P, SLAB], f32, tag="agg_ps")
                    if layer == 1:
                        nc.vector.memset(acc[:], 0.0)
                    for qq in ([qsel] if qsel is not None else range(NQ)):
                        k0, k1 = group_of[(s, qq)]
                        kn = k1 - k0
                        if layer == 0:
                            gt = gpool.tile([P, kn, P], f16, tag="gt")
                            nc.sync.dma_start(
                                out=gt[:], in_=g1[:, k0:k1, :])
                        else:
                            ist = gpool.tile([P, kn * P // 16], mybir.dt.int16,
                                             tag="iststage")
                            nc.sync.dma_start(
                                out=ist[:],
                                in_=idxs[:, k0 * P // 16: k1 * P // 16])
                            gt = gpool.tile([P, kn, P], f16, tag="gt")
                            ni = kn * P
                            nc.gpsimd.dma_gather(
                                gt[:], h1_tab[qq][:], ist[:],
                                ni, ni, P, single_packet=False,
                                queue_num=(s + qq) % 4)
                        # P builds + matmuls
                        k = k0
                        while k < k1:
                            kind = prog[k][2]
                            if kind in ("wide", "ovfl"):
                                if layer == 0:
                                    last = (qq == NQ - 1 and k == k1 - 1)
                                    first = (kind == "wide")
                                else:
                                    last = (k == k1 - 1)
                                    first = False
                                pm = ppool.tile([P, SLAB], f16, tag="pwide")
                                nc.vector.tensor_scalar(
                                    out=pm[:], in0=iota_w[:],
                                    scalar1=off_sb[:, k:k + 1],
                                    scalar2=w_sb[:, k:k + 1],
                                    op0=mybir.AluOpType.is_equal,
                                    op1=mybir.AluOpType.mult)
                                nc.tensor.matmul(
                                    out=acc[:], lhsT=gt[:, k - k0, :], rhs=pm[:],
                                    start=first, stop=last)
                                k += 1
                            else:
                                nb = 1
                                while (nb < PBATCH and k + nb < k1
                                       and prog[k + nb][2] == "norm"):
                                    nb += 1
                                pm = ppool.tile([P, nb, WIN], f16,
                                                tag=f"pn{nb}")
                                bco = bass.AP(
                                    off16.tensor, off16[:, k:k + nb].offset,
                                    [list(off16[:, k:k + nb].ap[0]),
                                     list(off16[:, k:k + nb].ap[1]),
                                     [0, WIN]])
                                bcw = bass.AP(
                                    w16.tensor, w16[:, k:k + nb].offset,
                                    [list(w16[:, k:k + nb].ap[0]),
                                     list(w16[:, k:k + nb].ap[1]),
                                     [0, WIN]])
                                nc.vector.tensor_tensor(
                                    out=pm[:], in0=iota_rep[:, :nb, :],
                                    in1=bco, op=mybir.AluOpType.is_equal)
                                nc.vector.tensor_tensor(
                                    out=pm[:], in0=pm[:], in1=bcw,
                                    op=mybir.AluOpType.mult)
                                for j in range(nb):
                                    base = prog[k + j][3]
                                    if layer == 0:
                                        last = (qq == NQ - 1 and k + j == k1 - 1)
                                    else:
                                        last = (k + j == k1 - 1)
                                    nc.tensor.matmul(
                                        out=acc[:, base:base + WIN],
                                        lhsT=gt[:, k + j - k0, :],
                                        rhs=pm[:, j, :],
                                        start=False, stop=last)
                                k += nb
                    if layer == 0:
                        nc.scalar.activation(
                            out=st_sb[:, wlo:wlo + wid], in_=acc[:, :wid],
                            func=mybir.ActivationFunctionType.Prelu,
                            alpha=alpha1_sb[:, 0:1])
                        return None
                    ev = epool.tile([P, SLAB], f16, tag="l2ev")
                    nc.scalar.activation(
                        out=ev[:, :wid], in_=acc[:, :wid],
                        func=mybir.ActivationFunctionType.Prelu,
                        alpha=alpha1_sb[:, 0:1])

                    def epilogue(s=s, qsel=qsel, ev=ev, wid=wid):
                        t0 = (s * SLAB) // P
                        for tt in range(t0, min(t0 + SLAB // P, ntile)):
                            c0 = tt * P - s * SLAB
                            m = min(P, nloc - tt * P)
                            ops = psg2.tile([P, 4], f32, tag="gemm2_ps")
                            nc.tensor.matmul(out=ops[:m, :3],
                                             lhsT=ev[:, c0:c0 + m],
                                             rhs=w2_sb[:], start=True,
                                             stop=(qsel != 0))
                            if qsel == 0:
                                nc.tensor.matmul(out=ops[:m, :3],
                                                 lhsT=ones_sb[:, :m],
                                                 rhs=b2_sb[:], start=False,
                                                 stop=True)
                            nc.vector.tensor_tensor(
                                out=out_acc[:m, tt, :3],
                                in0=out_acc[:m, tt, :3],
                                in1=ops[:m, :3], op=mybir.AluOpType.add)
                    return epilogue

            # ====== layer 1 + layer 2 interleaved emission ======
            # L2 (q,s) groups are emitted between later L1 slabs, as soon as
            # their quarter's AllGather is in the gpsimd stream, keeping every
            # engine's in-order stream dependency-ready.
            l2_queue = [(qq, s) for qq in range(NQ) for s in range(nslab)]
            l2_pos = 0
            pend_ep = []

            def emit_l2(qq2, s2):
                ep = agg_layer_slab(1, s2, qsel=qq2)
                pend_ep.append(ep)
                if len(pend_ep) > 2:
                    pend_ep.pop(0)()
            ag_emitted = 0
            done_tiles = 0
            for s in range(nslab):
                agg_layer_slab(0, s)
                cover = min(ntile_l, ((s + 1) * SLAB) // P) if s < nslab - 1 \
                    else ntile_l
                while done_tiles < cover:
                    gemm1_tile(done_tiles)
                    while ag_emitted < NQ and ag_after_tile[ag_emitted] == done_tiles:
                        do_allgather(ag_emitted)
                        ag_emitted += 1
                    done_tiles += 1
                # pace L2 groups whose table is already in flight
                budget = 3
                while (budget > 0 and l2_pos < len(l2_queue)
                       and l2_queue[l2_pos][0] < ag_emitted):
                    qq2, s2 = l2_queue[l2_pos]
                    emit_l2(qq2, s2)
                    l2_pos += 1
                    budget -= 1
            assert done_tiles == ntile_l and ag_emitted == NQ
            while l2_pos < len(l2_queue):
                qq2, s2 = l2_queue[l2_pos]
                emit_l2(qq2, s2)
                l2_pos += 1
            for ep in pend_ep:
                ep()

            for t in range(ntile):
                r0 = t * P
                m = min(P, nloc - r0)
                nc.sync.dma_start(out=out_t[r0:r0 + m, :],
                                  in_=out_acc[:m, t, :3])

    nc.compile()
    return nc


# ---------------------------------------------------------------------------
# entry point
# ---------------------------------------------------------------------------

def _wrap_idx(vals):
    """int16 gather index layout: slot i -> [i%16, i//16], tiled to 128 rows,
    built per 16-slot column group (layout wraps within each dma_gather call,
    which always covers a whole number of 16-slot columns)."""
    v = vals.reshape(-1, 16).T  # [16, n/16]
    return np.tile(v, (8, 1)).astype(np.int16)


def kernel(x, W1, b1, W2, b2, edge_index, _profile=None):
    import ml_dtypes
    bf16 = ml_dtypes.bfloat16
    x = np.asarray(x, np.float32)
    x16 = x.astype(bf16)
    meta = host_prep(x, edge_index)
    nloc, nch = meta["nloc"], meta["nch"]

    from concourse.bass_utils import run_bass_kernel_spmd
    nc = build_program(meta)

    in_maps = []
    for c in range(NCORES):
        pc = meta["per_core"][c]
        g1 = np.ascontiguousarray(
            x16[pc["slots_src"].reshape(nch, P)].transpose(1, 0, 2))
        in_maps.append(dict(
            g1=g1,
            idxs=_wrap_idx(pc["a_idx"]),
            dstoff=np.ascontiguousarray(pc["a_off"].T),
            dprod=np.ascontiguousarray(pc["a_dpr"].T),
            w1=np.asarray(W1, np.float32).astype(bf16),
            b1=np.asarray(b1, np.float32).reshape(1, P).astype(bf16),
            w2=np.asarray(W2, np.float32).astype(bf16),
            b2=np.asarray(b2, np.float32).reshape(1, 3).astype(bf16),
        ))

    res = run_bass_kernel_spmd(nc, in_maps, list(range(NCORES)),
                               trace=_profile is not None)
    if _profile is not None:
        _profile["exec_time_ns"] = res.exec_time_ns
    out = np.concatenate([res.results[c]["out"] for c in range(NCORES)], axis=0)
    return out.astype(np.float32)

